# revision 2
# baseline (speedup 1.0000x reference)
"""Trainium2 Bass kernel v3: transposed weight-stationary GRU + FC head.

Layouts (per core, batch BL=64):
  hT  [128, 384] fp16: col = k*64 + b   (k = h-dim chunk, b = batch)
  GALL [128, 1536] f32 PSUM (3 banks):
    [0:768]     Grz: col = j*64 + b   (j<6: r-chunk j, j>=6: z-chunk j-6)
    [768:1152]  Gin: col = 768 + k*64 + b
    [1152:1536] Ghn: col = 1152 + k*64 + b

Per step the PE runs 129 instructions (3 bias GEMMs N=512, 18 x-proj
N=64, 108 h-proj N=64 with 128x128 stationary weight blocks). The PE
sequencer costs ~20-30ns/instruction (measured; it, not the array, was
the v2 bottleneck at N=32), so N=64 balances SEQ (~3.9us) against the
array (~4.0us at 2.4GHz).

The elementwise tail is split into k-chunk halves (contiguous [128,192]
slices): half 0 (h-dims 0:384) is produced first and unblocks the next
step's k=0..2 sweeps while half 1 finishes, overlapping the tail with
PE work. Tail spread across ACT (sigmoids/tanh), DVE (tn/tn2/w1/h16)
and GpSimd (u, shadow h32).
"""

import os
import sys

import numpy as np

if "/opt/trn_rl_repo" not in sys.path:
    sys.path.insert(0, "/opt/trn_rl_repo")

B, T, I, H, F1, C = 512, 128, 128, 768, 256, 10
NCORES = 8
BL = B // NCORES  # 64
G3 = 3 * H  # 2304
KC = H // 128  # 6
NCH = G3 // 128  # 18

_CACHE = {}


def _build_program(reps=1):
    import concourse.bacc as bacc
    import concourse.mybir as mybir
    import concourse.tile as tile

    f16 = mybir.dt.float16
    f32 = mybir.dt.float32
    AF = mybir.ActivationFunctionType
    OP = mybir.AluOpType

    nc = bacc.Bacc(
        "TRN2",
        target_bir_lowering=False,
        debug=False,
        enable_asserts=False,
        num_devices=NCORES,
    )

    xT_d = nc.dram_tensor("xT", [128, T * BL], f16, kind="ExternalInput")
    whhT_d = nc.dram_tensor("whhT", [128, KC * G3], f16, kind="ExternalInput")
    wihT_d = nc.dram_tensor("wihT", [128, G3], f16, kind="ExternalInput")
    BT_d = nc.dram_tensor("BT", [24, 128], f16, kind="ExternalInput")
    IB_d = nc.dram_tensor("IB", [24, 1536], f16, kind="ExternalInput")
    wfc1T_d = nc.dram_tensor("wfc1T", [128, KC * F1], f16, kind="ExternalInput")
    bf1T_d = nc.dram_tensor("bf1T", [2, 128], f16, kind="ExternalInput")
    ifc1_d = nc.dram_tensor("ifc1", [2, 128], f16, kind="ExternalInput")
    wfc2T_d = nc.dram_tensor("wfc2T", [128, 2 * C], f16, kind="ExternalInput")
    bf2_d = nc.dram_tensor("bf2", [1, C], f16, kind="ExternalInput")
    ones64_d = nc.dram_tensor("ones64", [1, BL], f16, kind="ExternalInput")
    out_d = nc.dram_tensor("logitsT", [C, BL], f32, kind="ExternalOutput")

    # Per-half PSUM tile H_s [128, 768] (half s = h-chunks 3s..3s+2):
    #   cols j*64      : r gate, local chunk j
    #   192 + j*64     : z gate
    #   384 + j*64     : gin (x-proj n-part)
    #   576 + j*64     : ghn (h-proj n-part)
    # Separate tiles per half so the half-0 tail only waits on H_0
    # (PSUM dependency tracking is tile-granular).
    R0, Z0, IN0, HN0 = 0, 192, 384, 576

    with tile.TileContext(nc) as tc:
        with (
            tc.tile_pool(name="const", bufs=1) as const,
            tc.tile_pool(name="state", bufs=2) as state,
            tc.tile_pool(name="work", bufs=2) as work,
            tc.tile_pool(name="gpsum", bufs=1, space="PSUM") as gpsum,
        ):
            def load(name, shape, dram):
                t_ = const.tile(shape, f16, tag=name)
                nc.sync.dma_start(out=t_[:], in_=dram.ap())
                return t_

            xT = load("xT", [128, T * BL], xT_d)
            whhT = load("whhT", [128, KC * G3], whhT_d)
            wihT = load("wihT", [128, G3], wihT_d)
            BT = load("BT", [24, 128], BT_d)
            IB = load("IB", [24, 1536], IB_d)
            wfc1T = load("wfc1T", [128, KC * F1], wfc1T_d)
            bf1T = load("bf1T", [2, 128], bf1T_d)
            ifc1 = load("ifc1", [2, 128], ifc1_d)
            wfc2T = load("wfc2T", [128, 2 * C], wfc2T_d)
            bf2 = load("bf2", [1, C], bf2_d)
            ones64 = load("ones64", [1, BL], ones64_d)

            st8 = {"h16": None}

            def mm(out, lhsT, rhs, start, stop):
                nc.tensor.matmul(out, lhsT, rhs, start=start, stop=stop,
                                 skip_group_check=True)

            def gregion(Hs, s, c):
                # global gate chunk c -> slice of this half's tile
                if c < 6:  # r chunk c (half c//3, local c%3)
                    base = R0 + (c % 3) * 64
                elif c < 12:  # z chunk c-6
                    base = Z0 + ((c - 6) % 3) * 64
                else:  # ghn chunk c-12
                    base = HN0 + ((c - 12) % 3) * 64
                return Hs[:, base : base + 64]

            def emit_step(t):
                # four rotating PSUM tiles (half x step parity): a tile's
                # next writer comes 2 steps after its tail readers, so the
                # start=True bias never waits on the previous tail (WAR).
                p = t % 2
                H = [gpsum.tile([128, 768], f32, tag=f"H{s}{p}", name=f"H{s}{p}")
                     for s in (0, 1)]

                for s in (0, 1):
                    for c0 in (0, 512):
                        w = 512 if c0 == 0 else 256
                        mm(H[s][:, c0 : c0 + w], BT[:],
                           IB[:, s * 768 + c0 : s * 768 + c0 + w],
                           start=True, stop=False)

                h16p = st8["h16"]
                xt = xT[:, t * BL : (t + 1) * BL]

                # gate chunks belonging to half s: r 3s..3s+2, z +6, hn +12
                def half_gchunks(s):
                    return (3 * s, 3 * s + 12, 3 * s + 6,
                            3 * s + 1, 3 * s + 13, 3 * s + 7,
                            3 * s + 2, 3 * s + 14, 3 * s + 8)

                def xmm(s, c, stop):
                    o = gregion(H[s], s, c)
                    mm(o, wihT[:, c * 128 : (c + 1) * 128], xt,
                       start=False, stop=stop)
                    return o

                def ginmm(s, k, stop):
                    o = H[s][:, IN0 + (k % 3) * 64 : IN0 + (k % 3) * 64 + 64]
                    mm(o, wihT[:, (12 + k) * 128 : (13 + k) * 128], xt,
                       start=False, stop=stop)

                if t == 0:
                    for s in (0, 1):
                        for c in half_gchunks(s):
                            if c < 12:
                                xmm(s, c, stop=True)
                        for k in (3 * s, 3 * s + 1, 3 * s + 2):
                            ginmm(s, k, stop=True)
                else:
                    # A-blocks (x + k0..k2) consume the prior tail-H0
                    # pieces (early); B-blocks (k3..k5) consume the prior
                    # tail-H1 pieces, which land ~2.3us into the step. So:
                    # A_H0, then the first A_H1 chunks as filler, B_H0 at
                    # ~2.4us, rest of A_H1, B_H1. H0's gates complete at
                    # ~3.3us and their tail overlaps the rest of the step;
                    # H1's tail overlaps the next step's A-blocks.
                    def ablock(s, chunks):
                        # x + k0 for every chunk first, then all k1, then
                        # all k2: pushes the k1/k2 consumers later so the
                        # prior tail-H0's piece deliveries have more slack.
                        for c in chunks:
                            if c < 12:
                                xmm(s, c, stop=False)
                        for k in range(3):
                            for c in chunks:
                                o = gregion(H[s], s, c)
                                mm(o, whhT[:, k * G3 + c * 128 : k * G3 + (c + 1) * 128],
                                   h16p[:, k * 64 : (k + 1) * 64],
                                   start=False, stop=False)

                    def bblock(s):
                        for k in range(3, KC):
                            hk = h16p[:, k * 64 : (k + 1) * 64]
                            for c in half_gchunks(s):
                                mm(gregion(H[s], s, c),
                                   whhT[:, k * G3 + c * 128 : k * G3 + (c + 1) * 128],
                                   hk, start=False, stop=(k == KC - 1))

                    for k in (0, 1, 2):
                        ginmm(0, k, stop=True)
                    ablock(0, half_gchunks(0))
                    for k in (3, 4, 5):
                        ginmm(1, k, stop=True)
                    ablock(1, half_gchunks(1)[:4])
                    bblock(0)
                    ablock(1, half_gchunks(1)[4:])
                    bblock(1)

                # tail: two 192-wide half chains
                #   sig r (ACT) -> tn (DVE) -> tn2 (DVE) -> tanh (ACT)
                #   -> w1 (DVE stt) -> h16 (DVE, 3x 64-wide for progressive
                #   delivery); z-sigmoid + u = z*h_prev shadow on ACT/GpSimd.
                r32 = work.tile([128, 384], f32, tag="r")
                z32 = work.tile([128, 384], f32, tag="z")
                tn = work.tile([128, 384], f32, tag="tn")
                tn2 = work.tile([128, 384], f32, tag="tn2")
                n32 = work.tile([128, 384], f32, tag="n")
                u32 = work.tile([128, 384], f32, tag="u")
                w1 = work.tile([128, 384], f32, tag="w1")
                h16 = state.tile([128, 384], f16, tag="h16")
                h16p_s = st8["h16"]

                def half_tail(s):
                    # chain: sig r (ACT) -> tn,tn2 (DVE) -> tanh (ACT) ->
                    # w1 (DVE stt) -> h16 (j0/j2 DVE, j1 GpSimd); u = z*h
                    # shadow on GpSimd.
                    sl = slice(s * 192, s * 192 + 192)
                    nc.scalar.activation(r32[:, sl], H[s][:, R0 : R0 + 192],
                                         AF.Sigmoid)
                    nc.scalar.activation(z32[:, sl], H[s][:, Z0 : Z0 + 192],
                                         AF.Sigmoid)
                    nc.vector.tensor_mul(tn[:, sl], r32[:, sl],
                                         H[s][:, HN0 : HN0 + 192])
                    nc.vector.tensor_add(tn2[:, sl], tn[:, sl],
                                         H[s][:, IN0 : IN0 + 192])
                    nc.scalar.activation(n32[:, sl], tn2[:, sl], AF.Tanh)
                    if t > 0:
                        nc.gpsimd.tensor_mul(u32[:, sl], z32[:, sl],
                                             h16p_s[:, sl])
                    nc.vector.scalar_tensor_tensor(
                        w1[:, sl], z32[:, sl], 1.0, n32[:, sl],
                        op0=OP.subtract, op1=OP.mult)
                    for j, eng in ((0, nc.vector), (1, nc.gpsimd),
                                   (2, nc.vector)):
                        jsl = slice(s * 192 + j * 64, s * 192 + (j + 1) * 64)
                        if t > 0:
                            eng.tensor_sub(h16[:, jsl], u32[:, jsl],
                                           w1[:, jsl])
                        else:
                            eng.tensor_scalar_mul(h16[:, jsl], w1[:, jsl],
                                                  -1.0)

                half_tail(0)
                half_tail(1)
                st8["h16"] = h16

            def emit_fc_head():
                h16 = st8["h16"]
                F1t = gpsum.tile([128, 768], f32, tag="H00", name="F1t")
                F1ps = F1t[:, 0:128]
                mm(F1ps[:], bf1T[:], ifc1[:], start=True, stop=False)
                for f in range(2):
                    for k in range(KC):
                        w = wfc1T[:, k * F1 + f * 128 : k * F1 + (f + 1) * 128]
                        mm(F1ps[:, f * 64 : (f + 1) * 64], w,
                           h16[:, k * 64 : (k + 1) * 64],
                           start=False, stop=(k == KC - 1))
                o1T = work.tile([128, 128], f16, tag="o1T")
                nc.scalar.activation(o1T[:], F1ps[:], AF.Relu)

                C2t = gpsum.tile([128, 768], f32, tag="H10", name="C2t")
                C2ps = C2t[0:C, 0:BL]
                mm(C2ps[:], bf2[:], ones64[:], start=True, stop=False)
                for j in range(2):
                    mm(C2ps[:], wfc2T[:, j * C : (j + 1) * C],
                       o1T[:, j * 64 : (j + 1) * 64],
                       start=False, stop=(j == 1))
                lo = work.tile([C, BL], f32, tag="lo")
                nc.vector.tensor_copy(lo[:], C2ps[:])
                nc.sync.dma_start(out=out_d.ap(), in_=lo[:])

            def emit_body():
                for t in range(T):
                    emit_step(t)
                emit_fc_head()

            if reps > 1:
                with tc.For_i(0, reps, 1):
                    emit_body()
            else:
                emit_body()

    nc.compile()
    return nc


def _prep_shared(w_ih, w_hh, b_ih, b_hh, w_fc1, b_fc1, w_fc2, b_fc2):
    f16 = np.float16

    def kmajor(wT, kc, n):
        return np.ascontiguousarray(
            wT.reshape(kc, 128, n).transpose(1, 0, 2).reshape(128, kc * n)
        )

    whhT = kmajor(np.ascontiguousarray(w_hh.T), KC, G3).astype(f16)
    wihT = np.ascontiguousarray(w_ih.T).astype(f16)

    brz = (b_ih + b_hh)[: 2 * H].astype(np.float32)
    BT = np.concatenate(
        [brz.reshape(12, 128),
         b_ih[2 * H :].reshape(6, 128).astype(np.float32),
         b_hh[2 * H :].reshape(6, 128).astype(np.float32)],
        axis=0,
    ).astype(f16)
    # IB columns: [H0 tile 768 | H1 tile 768], tile layout
    # [r(3x64) | z(3x64) | gin(3x64) | ghn(3x64)]
    IB = np.zeros((24, 1536), dtype=f16)
    for s in range(2):
        for j in range(3):
            g = 3 * s + j  # global h-chunk / rz-chunk index
            IB[g, s * 768 + j * 64 : s * 768 + (j + 1) * 64] = 1.0  # r
            IB[6 + g, s * 768 + 192 + j * 64 : s * 768 + 192 + (j + 1) * 64] = 1.0
            IB[12 + g, s * 768 + 384 + j * 64 : s * 768 + 384 + (j + 1) * 64] = 1.0
            IB[18 + g, s * 768 + 576 + j * 64 : s * 768 + 576 + (j + 1) * 64] = 1.0

    wfc1T = kmajor(np.ascontiguousarray(w_fc1.T), KC, F1).astype(f16)
    bf1T = b_fc1.reshape(2, 128).astype(f16)
    ifc1 = np.zeros((2, 128), dtype=f16)
    ifc1[0, 0:64] = 1.0
    ifc1[1, 64:128] = 1.0
    wfc2T = kmajor(np.ascontiguousarray(w_fc2.T), 2, C).astype(f16)
    bf2 = b_fc2.reshape(1, C).astype(f16)
    ones64 = np.ones((1, BL), dtype=f16)
    return {
        "whhT": whhT, "wihT": wihT, "BT": BT, "IB": IB,
        "wfc1T": wfc1T, "bf1T": bf1T, "ifc1": ifc1,
        "wfc2T": wfc2T, "bf2": bf2, "ones64": ones64,
    }


def _prep_in_maps(inputs):
    x = np.asarray(inputs["x"], dtype=np.float32)
    shared = _prep_shared(
        *(np.asarray(inputs[k], dtype=np.float32)
          for k in ("w_ih", "w_hh", "b_ih", "b_hh", "w_fc1", "b_fc1",
                    "w_fc2", "b_fc2"))
    )
    in_maps = []
    for c in range(NCORES):
        xs = x[c * BL : (c + 1) * BL]
        xT = np.ascontiguousarray(
            xs.transpose(2, 1, 0).reshape(128, T * BL)
        ).astype(np.float16)
        in_maps.append({**shared, "xT": xT})
    return in_maps


def _execute(in_maps, reps=1):
    from concourse.bass_utils import run_bass_kernel_spmd

    key = ("nc", reps)
    if key not in _CACHE:
        _CACHE[key] = _build_program(reps=reps)
    nc = _CACHE[key]
    res = run_bass_kernel_spmd(nc, in_maps, core_ids=list(range(NCORES)))
    out = np.concatenate(
        [res.results[c]["logitsT"].T for c in range(NCORES)], axis=0
    )
    return np.ascontiguousarray(out).astype(np.float32), res


def kernel(**inputs):
    out, _ = _execute(_prep_in_maps(inputs))
    return out
